# revision 19
# baseline (speedup 1.0000x reference)
"""MoE actor (16 experts, top-4) Trainium2 kernel, data-parallel over 8 NeuronCores.

Math per token t:
    logits = x @ router_w.T + router_b             [E]
    probs  = softmax(logits)
    sp     = probs * topk4_mask(logits)            [E]  (masked, not renormalized)
    mean   = sum_e sp[e] * (x @ mean_w[e].T    + mean_b[e])
    lstd   = sum_e sp[e] * (x @ log_std_w[e].T + log_std_b[e])
    lstd   = 1.75 * tanh(lstd) - 3.25

Device strategy (per core, T=2048 tokens; all matmuls bf16 at N=512):
  - x arrives transposed as an exact hi/lo bf16 pair (xh + xl == f32 x);
    expert weights arrive as one concatenated stack wcat[o, e*512+a]
    (mean|log_std along a, 512 wide).
  - Router per 512-token chunk: logitsT [16, chunk] from 8 matmuls with a
    packed [128, 48] stationary operand (rwh | zero gap | rwl; the gap keeps
    lo rows PSUM-32-aligned) against xh and xl — exact-to-2nd-order logits,
    so top-4 selection matches the f32 reference. Plain K=16 matmuls vs a
    16x16 identity transpose logits to token-major (transpose_mode with 16
    partitions crashes the device). Softmax + max8-threshold top-4 masking
    run batched on [128, 64] DVE/ACT ops; sp -> PE-transpose -> spT [16, T].
  - spT bounces through DRAM so each expert's gate row can be
    partition-broadcast-loaded as a [128, chunk] tile.
  - Main per chunk (o-outer so matmuls never wait on later weight tiles):
    scale xh tiles by the broadcast gate row (DVE), then 16 experts x 16
    matmuls + a K=16 bias matmul accumulate into 4 PSUM banks =
    outT[512, chunk] f32.
  - a-rows 256..511 are log_std: tanh (ACT) + affine (DVE) before store.

No collectives: pure SPMD data parallelism; host shards/gathers.
"""

from contextlib import ExitStack

import ml_dtypes
import numpy as np

import concourse.bass as bass
import concourse.mybir as mybir
import concourse.tile as tile
from concourse import bacc
from concourse.bass_utils import run_bass_kernel_spmd
from concourse.masks import make_identity

BF16 = mybir.dt.bfloat16
F32 = mybir.dt.float32
NP_BF16 = ml_dtypes.bfloat16

P = 128
NCORES = 8
B_FULL = 16384
OBS = 512
ACT_DIM = 256
E = 16
A2 = 2 * ACT_DIM  # 512: mean|log_std concatenated
OT = OBS // P     # 4 o-tiles

LOG_STD_SCALE = 3.5   # 0.5*(LOG_STD_MAX-LOG_STD_MIN)
LOG_STD_SHIFT = -1.5  # LOG_STD_MIN + 0.5*(MAX-MIN)


def build_nc(T):
    """Build the single-core Bacc program for a T-token shard."""
    # uniform 512-token chunks: N=512 matmuls fully hide LDWEIGHTS; smaller
    # chunks measured slower (N=128 matmuls become weight-load-bound).
    CH_SIZES = []
    r = T
    while r > 0:
        s = min(512, r)
        CH_SIZES.append(s)
        r -= s
    assert sum(CH_SIZES) == T and all(s % P == 0 for s in CH_SIZES)
    CH_STARTS = [sum(CH_SIZES[:i]) for i in range(len(CH_SIZES))]
    NTC = len(CH_SIZES)
    TCH = max(CH_SIZES)

    nc = bacc.Bacc("TRN2", target_bir_lowering=False, debug=False)

    xh_d = nc.declare_dram_parameter("xh", [OBS, T], BF16, isOutput=False)
    xl_d = nc.declare_dram_parameter("xl", [OBS, T], BF16, isOutput=False)
    wcat_d = nc.declare_dram_parameter("wcat", [OBS, E * A2], BF16, isOutput=False)
    bcat_d = nc.declare_dram_parameter("bcat", [E, A2], BF16, isOutput=False)
    rwT_d = nc.declare_dram_parameter("rwT", [OBS, E], F32, isOutput=False)
    rb_d = nc.declare_dram_parameter("rb", [1, E], F32, isOutput=False)
    outT_d = nc.declare_dram_parameter("outT", [A2, T], F32, isOutput=True)

    with tile.TileContext(nc) as tc, ExitStack() as ctx:
        wpool = ctx.enter_context(tc.tile_pool(name="weights", bufs=1))
        dpool = ctx.enter_context(tc.tile_pool(name="spd", bufs=1, space="DRAM"))
        rpsum = ctx.enter_context(tc.tile_pool(name="rpsum", bufs=1, space="PSUM"))
        tpsum = ctx.enter_context(tc.tile_pool(name="tpsum", bufs=2, space="PSUM"))
        rsb = ctx.enter_context(tc.tile_pool(name="rsb", bufs=3))
        mpsum = ctx.enter_context(tc.tile_pool(name="mpsum", bufs=1, space="PSUM"))
        srpool = ctx.enter_context(tc.tile_pool(name="srep", bufs=2 * E + 2))
        rspool = ctx.enter_context(tc.tile_pool(name="rs", bufs=8))
        opool = ctx.enter_context(tc.tile_pool(name="outb", bufs=3))

        # --- loads, in router-first order so PE can start within ~3us ---
        RW = []
        for o in range(OT):
            rwt = wpool.tile([P, E], F32, tag=f"rw{o}")
            nc.sync.dma_start(rwt[:], rwT_d[o * P:(o + 1) * P, :])
            RW.append(rwt)
        # x as an exact hi/lo bf16 pair (host-split: xh+xl == f32 x),
        # chunked by token-chunk so chunk 0 lands first
        X = [[None] * NTC for _ in range(OT)]
        Xl = [[None] * NTC for _ in range(OT)]
        for tci in range(NTC):
            c0, cs = CH_STARTS[tci], CH_SIZES[tci]
            for o in range(OT):
                t = wpool.tile([P, cs], BF16, tag=f"x{o}_{tci}",
                               name=f"x{o}_{tci}")
                nc.sync.dma_start(t[:], xh_d[o * P:(o + 1) * P, c0:c0 + cs])
                X[o][tci] = t
                tl = wpool.tile([P, cs], BF16, tag=f"xl{o}_{tci}",
                                name=f"xl{o}_{tci}")
                nc.sync.dma_start(tl[:], xl_d[o * P:(o + 1) * P, c0:c0 + cs])
                Xl[o][tci] = tl
        RBB = wpool.tile([P, E], F32, tag="rbb")
        nc.sync.dma_start(RBB[:], rb_d[0:1, :].to_broadcast([P, E]))
        ident = wpool.tile([P, P], F32, tag="ident")
        make_identity(nc, ident[:])
        Bc = wpool.tile([E, A2], BF16, tag="bc")
        nc.sync.dma_start(Bc[:], bcat_d[:, :])
        # router weights split hi/lo bf16 (rw = rwh + rwl to ~1.6e-5), packed
        # into one [128, 48] stationary operand: cols 0:16 = rwh, 32:48 = rwl
        # (the 16-col zero gap keeps the lo output rows 32-aligned in PSUM).
        RW2 = []
        for o in range(OT):
            h2 = wpool.tile([P, 3 * E], BF16, tag=f"rw2_{o}")
            nc.vector.memset(h2[:], 0)
            nc.vector.tensor_copy(h2[:, 0:E], RW[o][:])
            lf = wpool.tile([P, E], F32, tag=f"rwlf{o}")
            nc.vector.tensor_copy(h2[:, 2 * E:3 * E], RW[o][:])
            nc.vector.tensor_sub(lf[:], RW[o][:], h2[:, 2 * E:3 * E])
            nc.vector.tensor_copy(h2[:, 2 * E:3 * E], lf[:])
            RW2.append(h2)

        W = []
        for o in range(OT):
            wt = wpool.tile([P, E * A2], BF16, tag=f"w{o}")
            nc.sync.dma_start(wt[:], wcat_d[o * P:(o + 1) * P, :])
            W.append(wt)
        spT = wpool.tile([E, T], BF16, tag="spt")
        spd = dpool.tile([E, T], BF16, tag="spd")

        LTT = T // P  # all token tiles
        lgbF = wpool.tile([P, LTT * E], F32, tag="lgbF")
        spvF = wpool.tile([P, LTT * E], F32, tag="spvF")

        srep_by_chunk = {}

        def routerA_chunk(tci):
            c0, cs = CH_STARTS[tci], CH_SIZES[tci]
            TILES_PER_CH = cs // P
            LE = TILES_PER_CH * E
            lt0 = c0 // P
            plT = rpsum.tile([3 * E, cs], F32, tag="rplt", name=f"plT{tci}")
            nmm = 2 * OT
            i = 0
            for o in range(OT):
                for rh in (X[o][tci], Xl[o][tci]):
                    nc.tensor.matmul(plT[:], lhsT=RW2[o][:], rhs=rh[:],
                                     start=(i == 0), stop=(i == nmm - 1))
                    i += 1
            lgTl = rsb.tile([E, cs], F32, tag="lgTl")
            nc.scalar.copy(lgTl[:], plT[2 * E:3 * E, :])
            lgT = rsb.tile([E, cs], F32, tag="lgT")
            nc.vector.tensor_add(lgT[:], plT[0:E, :], lgTl[:])
            pl = rpsum.tile([P, LE], F32, tag="rpsum")
            for lt in range(TILES_PER_CH):
                nc.tensor.matmul(pl[:, lt * E:(lt + 1) * E],
                                 lhsT=lgT[:, lt * P:(lt + 1) * P],
                                 rhs=ident[:E, :E],
                                 start=True, stop=True)
            rbb3 = RBB[:].rearrange("p (l e) -> p l e", l=1).to_broadcast(
                [P, TILES_PER_CH, E])
            nc.vector.tensor_tensor(
                lgbF[:, lt0 * E:(lt0 + TILES_PER_CH) * E]
                .rearrange("p (l e) -> p l e", e=E),
                pl[:].rearrange("p (l e) -> p l e", e=E), rbb3,
                op=mybir.AluOpType.add)

        def router_batch():
            lg3 = lgbF[:].rearrange("p (l e) -> p l e", e=E)
            mx = rsb.tile([P, LTT], F32, tag="mx")
            nc.vector.reduce_max(mx[:], lg3, axis=mybir.AxisListType.X)
            mxb = mx[:].rearrange("p (l e) -> p l e", e=1).to_broadcast(
                [P, LTT, E])
            lgs = rsb.tile([P, LTT * E], F32, tag="lgs")
            nc.vector.tensor_sub(
                lgs[:].rearrange("p (l e) -> p l e", e=E), lg3, mxb)
            ex = rsb.tile([P, LTT * E], F32, tag="ex")
            nc.scalar.activation(ex[:], lgs[:],
                                 mybir.ActivationFunctionType.Exp)
            den = rsb.tile([P, LTT], F32, tag="den")
            nc.vector.reduce_sum(den[:],
                                 ex[:].rearrange("p (l e) -> p l e", e=E),
                                 axis=mybir.AxisListType.X)
            rden = rsb.tile([P, LTT], F32, tag="rden")
            nc.vector.reciprocal(rden[:], den[:])
            t8b = rsb.tile([P, 8 * LTT], F32, tag="t8b")
            for lt in range(LTT):
                nc.vector.max(out=t8b[:, lt * 8:(lt + 1) * 8],
                              in_=lgbF[:, lt * E:(lt + 1) * E])
            thrb = t8b[:].rearrange("p (l k) -> p l k", k=8)[:, :, 3:4] \
                .to_broadcast([P, LTT, E])
            mask = rsb.tile([P, LTT * E], F32, tag="mask")
            nc.vector.tensor_tensor(
                mask[:].rearrange("p (l e) -> p l e", e=E), lg3, thrb,
                op=mybir.AluOpType.is_ge)
            spm = rsb.tile([P, LTT * E], F32, tag="spm")
            nc.vector.tensor_mul(spm[:], ex[:], mask[:])
            rdenb = rden[:].rearrange("p (l e) -> p l e", e=1).to_broadcast(
                [P, LTT, E])
            nc.vector.tensor_tensor(
                spvF[:].rearrange("p (l e) -> p l e", e=E),
                spm[:].rearrange("p (l e) -> p l e", e=E), rdenb,
                op=mybir.AluOpType.mult)

        def routerC_chunk(tci):
            c0, cs = CH_STARTS[tci], CH_SIZES[tci]
            TILES_PER_CH = cs // P
            lt0 = c0 // P
            ccols = slice(c0, c0 + cs)
            for lt in range(TILES_PER_CH):
                t0 = c0 + lt * P
                g = lt0 + lt
                pt = tpsum.tile([E, P], F32, tag="tpsum",
                                name=f"pt{tci}_{lt}")
                nc.tensor.transpose(pt[:], spvF[:, g * E:(g + 1) * E],
                                    ident[:])
                nc.vector.tensor_copy(spT[:, t0:t0 + P], pt[:])
            nc.scalar.dma_start(spd[:, ccols], spT[:, ccols])
            sreps = []
            for e in range(E):
                srep = srpool.tile([P, cs], BF16, tag="srep",
                                   name=f"srep{e}_{tci}")
                nc.scalar.dma_start(srep[:],
                                    spd[e:e + 1, ccols].to_broadcast([P, cs]))
                sreps.append(srep)
            srep_by_chunk[tci] = sreps

        def main_chunk(tci):
            c0, cs = CH_STARTS[tci], CH_SIZES[tci]
            ccols = slice(c0, c0 + cs)
            sreps = srep_by_chunk[tci]
            # ---------------- expert accumulation (o-outer) ----------------
            ps = [mpsum.tile([P, cs], F32, tag=f"ps{a}",
                             name=f"ps{a}_{tci}") for a in range(4)]
            for a in range(4):
                nc.tensor.matmul(ps[a][:], lhsT=Bc[:, a * P:(a + 1) * P],
                                 rhs=spT[:, ccols], start=True, stop=False)
            for o in range(OT):
                for e in range(E):
                    r = rspool.tile([P, cs], BF16, tag="rs",
                                    name=f"rs{o}_{e}_{tci}")
                    nc.vector.tensor_mul(r[:], X[o][tci][:], sreps[e][:])
                    for a in range(4):
                        nc.tensor.matmul(
                            ps[a][:],
                            lhsT=W[o][:, e * A2 + a * P: e * A2 + (a + 1) * P],
                            rhs=r[:],
                            start=False,
                            stop=(o == OT - 1 and e == E - 1),
                        )
            for a in range(2):
                ob = opool.tile([P, cs], F32, tag="ob")
                nc.scalar.copy(ob[:], ps[a][:])
                nc.sync.dma_start(outT_d[a * P:(a + 1) * P, ccols], ob[:])
            for a in range(2, 4):
                th = opool.tile([P, cs], F32, tag="th")
                nc.scalar.activation(th[:], ps[a][:],
                                     mybir.ActivationFunctionType.Tanh)
                ob = opool.tile([P, cs], F32, tag="ob")
                nc.vector.tensor_scalar(ob[:], th[:], LOG_STD_SCALE,
                                        LOG_STD_SHIFT,
                                        op0=mybir.AluOpType.mult,
                                        op1=mybir.AluOpType.add)
                nc.sync.dma_start(outT_d[a * P:(a + 1) * P, ccols], ob[:])

        for i in range(NTC):
            routerA_chunk(i)
        router_batch()
        for i in range(NTC):
            routerC_chunk(i)
            main_chunk(i)

    nc.compile()
    return nc


def _host_prep(inputs, ncores=NCORES):
    x = np.asarray(inputs["x"], np.float32)
    rw = np.asarray(inputs["router_w"], np.float32)
    rb = np.asarray(inputs["router_b"], np.float32)
    mw = np.asarray(inputs["mean_w"], np.float32)
    mb = np.asarray(inputs["mean_b"], np.float32)
    lw = np.asarray(inputs["log_std_w"], np.float32)
    lb = np.asarray(inputs["log_std_b"], np.float32)

    B = x.shape[0]
    T = B // ncores

    # wcat[o, e*A2 + a] = (mean|log_std)_w[e, a, o]
    wc = np.concatenate([mw.transpose(0, 2, 1), lw.transpose(0, 2, 1)], axis=2)
    wcat = np.ascontiguousarray(wc.transpose(1, 0, 2)).reshape(OBS, E * A2)
    wcat = wcat.astype(NP_BF16)
    bcat = np.concatenate([mb, lb], axis=1).astype(NP_BF16)
    rwT = np.ascontiguousarray(rw.T).astype(np.float32)
    rbv = rb.reshape(1, E).astype(np.float32)

    shards = x.reshape(ncores, T, OBS)
    in_maps = []
    for c in range(ncores):
        xTf = np.ascontiguousarray(shards[c].T.astype(np.float32))
        xh = xTf.astype(NP_BF16)
        xl = (xTf - xh.astype(np.float32)).astype(NP_BF16)
        in_maps.append({"xh": xh, "xl": xl, "wcat": wcat, "bcat": bcat,
                        "rwT": rwT, "rb": rbv})
    return in_maps, T


_NC_CACHE = {}


def _get_nc(T):
    if T not in _NC_CACHE:
        _NC_CACHE[T] = build_nc(T)
    return _NC_CACHE[T]


def run_sharded(inputs, trace=False):
    """Returns ((mean, log_std), BassKernelResults)."""
    in_maps, T = _host_prep(inputs)
    nc = _get_nc(T)
    res = run_bass_kernel_spmd(nc, in_maps, list(range(NCORES)), trace=trace)
    outs = [res.results[c]["outT"] for c in range(NCORES)]
    mean = np.concatenate([o[:ACT_DIM].T for o in outs], axis=0)
    log_std = np.concatenate([o[ACT_DIM:].T for o in outs], axis=0)
    return (np.ascontiguousarray(mean, dtype=np.float32),
            np.ascontiguousarray(log_std, dtype=np.float32)), res


def kernel(**inputs):
    (mean, log_std), _ = run_sharded(inputs, trace=False)
    return mean, log_std



# revision 20
# speedup vs baseline: 1.0186x; 1.0186x over previous
"""MoE actor (16 experts, top-4) Trainium2 kernel, data-parallel over 8 NeuronCores.

Math per token t:
    logits = x @ router_w.T + router_b             [E]
    probs  = softmax(logits)
    sp     = probs * topk4_mask(logits)            [E]  (masked, not renormalized)
    mean   = sum_e sp[e] * (x @ mean_w[e].T    + mean_b[e])
    lstd   = sum_e sp[e] * (x @ log_std_w[e].T + log_std_b[e])
    lstd   = 1.75 * tanh(lstd) - 3.25

Device strategy (per core, T=2048 tokens; all matmuls bf16 at N=512):
  - x arrives transposed as an exact hi/lo bf16 pair (xh + xl == f32 x);
    expert weights arrive as one concatenated stack wcat[o, e*512+a]
    (mean|log_std along a, 512 wide).
  - Router per 512-token chunk: logitsT [16, chunk] from 8 matmuls with a
    packed [128, 48] stationary operand (rwh | zero gap | rwl; the gap keeps
    lo rows PSUM-32-aligned) against xh and xl — exact-to-2nd-order logits,
    so top-4 selection matches the f32 reference. Plain K=16 matmuls vs a
    16x16 identity transpose logits to token-major (transpose_mode with 16
    partitions crashes the device). Softmax + max8-threshold top-4 masking
    run batched on [128, 64] DVE/ACT ops; sp -> PE-transpose -> spT [16, T].
  - spT bounces through DRAM so each expert's gate row can be
    partition-broadcast-loaded as a [128, chunk] tile.
  - Main per chunk (o-outer so matmuls never wait on later weight tiles):
    scale xh tiles by the broadcast gate row (DVE), then 16 experts x 16
    matmuls + a K=16 bias matmul accumulate into 4 PSUM banks =
    outT[512, chunk] f32.
  - a-rows 256..511 are log_std: tanh (ACT) + affine (DVE) before store.

No collectives: pure SPMD data parallelism; host shards/gathers.
"""

from contextlib import ExitStack

import ml_dtypes
import numpy as np

import concourse.bass as bass
import concourse.mybir as mybir
import concourse.tile as tile
from concourse import bacc
from concourse.bass_utils import run_bass_kernel_spmd
from concourse.masks import make_identity

BF16 = mybir.dt.bfloat16
F32 = mybir.dt.float32
NP_BF16 = ml_dtypes.bfloat16

P = 128
NCORES = 8
B_FULL = 16384
OBS = 512
ACT_DIM = 256
E = 16
A2 = 2 * ACT_DIM  # 512: mean|log_std concatenated
OT = OBS // P     # 4 o-tiles

LOG_STD_SCALE = 3.5   # 0.5*(LOG_STD_MAX-LOG_STD_MIN)
LOG_STD_SHIFT = -1.5  # LOG_STD_MIN + 0.5*(MAX-MIN)


def build_nc(T):
    """Build the single-core Bacc program for a T-token shard."""
    # uniform 512-token chunks: N=512 matmuls fully hide LDWEIGHTS; smaller
    # chunks measured slower (N=128 matmuls become weight-load-bound).
    CH_SIZES = []
    r = T
    while r > 0:
        s = min(512, r)
        CH_SIZES.append(s)
        r -= s
    assert sum(CH_SIZES) == T and all(s % P == 0 for s in CH_SIZES)
    CH_STARTS = [sum(CH_SIZES[:i]) for i in range(len(CH_SIZES))]
    NTC = len(CH_SIZES)
    TCH = max(CH_SIZES)

    nc = bacc.Bacc("TRN2", target_bir_lowering=False, debug=False)

    xh_d = nc.declare_dram_parameter("xh", [OBS, T], BF16, isOutput=False)
    xl_d = nc.declare_dram_parameter("xl", [OBS, T], BF16, isOutput=False)
    wcat_d = nc.declare_dram_parameter("wcat", [OBS, E * A2], BF16, isOutput=False)
    bcat_d = nc.declare_dram_parameter("bcat", [E, A2], BF16, isOutput=False)
    rwT_d = nc.declare_dram_parameter("rwT", [OBS, E], F32, isOutput=False)
    rb_d = nc.declare_dram_parameter("rb", [1, E], F32, isOutput=False)
    outT_d = nc.declare_dram_parameter("outT", [A2, T], F32, isOutput=True)

    with tile.TileContext(nc) as tc, ExitStack() as ctx:
        wpool = ctx.enter_context(tc.tile_pool(name="weights", bufs=1))
        dpool = ctx.enter_context(tc.tile_pool(name="spd", bufs=1, space="DRAM"))
        rpsum = ctx.enter_context(tc.tile_pool(name="rpsum", bufs=1, space="PSUM"))
        tpsum = ctx.enter_context(tc.tile_pool(name="tpsum", bufs=2, space="PSUM"))
        rsb = ctx.enter_context(tc.tile_pool(name="rsb", bufs=3))
        mpsum = ctx.enter_context(tc.tile_pool(name="mpsum", bufs=1, space="PSUM"))
        srpool = ctx.enter_context(tc.tile_pool(name="srep", bufs=2 * E + 2))
        rspool = ctx.enter_context(tc.tile_pool(name="rs", bufs=8))
        opool = ctx.enter_context(tc.tile_pool(name="outb", bufs=3))

        # --- loads, in router-first order so PE can start within ~3us ---
        RW = []
        for o in range(OT):
            rwt = wpool.tile([P, E], F32, tag=f"rw{o}")
            nc.sync.dma_start(rwt[:], rwT_d[o * P:(o + 1) * P, :])
            RW.append(rwt)
        # x as an exact hi/lo bf16 pair (host-split: xh+xl == f32 x),
        # chunked by token-chunk so chunk 0 lands first
        X = [[None] * NTC for _ in range(OT)]
        Xl = [[None] * NTC for _ in range(OT)]
        for tci in range(NTC):
            c0, cs = CH_STARTS[tci], CH_SIZES[tci]
            for o in range(OT):
                t = wpool.tile([P, cs], BF16, tag=f"x{o}_{tci}",
                               name=f"x{o}_{tci}")
                nc.sync.dma_start(t[:], xh_d[o * P:(o + 1) * P, c0:c0 + cs])
                X[o][tci] = t
                tl = wpool.tile([P, cs], BF16, tag=f"xl{o}_{tci}",
                                name=f"xl{o}_{tci}")
                nc.sync.dma_start(tl[:], xl_d[o * P:(o + 1) * P, c0:c0 + cs])
                Xl[o][tci] = tl
        RBB = wpool.tile([P, E], F32, tag="rbb")
        nc.sync.dma_start(RBB[:], rb_d[0:1, :].to_broadcast([P, E]))
        ident = wpool.tile([P, P], F32, tag="ident")
        make_identity(nc, ident[:])
        Bc = wpool.tile([E, A2], BF16, tag="bc")
        nc.sync.dma_start(Bc[:], bcat_d[:, :])
        # router weights split hi/lo bf16 (rw = rwh + rwl to ~1.6e-5), packed
        # into one [128, 48] stationary operand: cols 0:16 = rwh, 32:48 = rwl
        # (the 16-col zero gap keeps the lo output rows 32-aligned in PSUM).
        RW2 = []
        for o in range(OT):
            h2 = wpool.tile([P, 3 * E], BF16, tag=f"rw2_{o}")
            nc.vector.memset(h2[:], 0)
            nc.vector.tensor_copy(h2[:, 0:E], RW[o][:])
            lf = wpool.tile([P, E], F32, tag=f"rwlf{o}")
            nc.vector.tensor_copy(h2[:, 2 * E:3 * E], RW[o][:])
            nc.vector.tensor_sub(lf[:], RW[o][:], h2[:, 2 * E:3 * E])
            nc.vector.tensor_copy(h2[:, 2 * E:3 * E], lf[:])
            RW2.append(h2)

        W = []
        for o in range(OT):
            wt = wpool.tile([P, E * A2], BF16, tag=f"w{o}")
            nc.sync.dma_start(wt[:], wcat_d[o * P:(o + 1) * P, :])
            W.append(wt)
        spT = wpool.tile([E, T], BF16, tag="spt")
        spd = dpool.tile([E, T], BF16, tag="spd")

        srep_by_chunk = {}

        def router_chunk(tci):
            c0, cs = CH_STARTS[tci], CH_SIZES[tci]
            TILES_PER_CH = cs // P
            LE = TILES_PER_CH * E  # logits columns per chunk (lt-major)
            ccols = slice(c0, c0 + cs)
            # ------- router for this chunk, batched over its 4 token tiles --
            # logitsT via hi/lo bf16 (xh@wh + xl@wh + xh@wl ~= f32 exact):
            # weights stationary (16-col LDWEIGHTS), tokens stream at N=cs.
            plT = rpsum.tile([3 * E, cs], F32, tag="rplt", name=f"plT{tci}")
            nmm = 2 * OT
            i = 0
            for o in range(OT):
                for rh in (X[o][tci], Xl[o][tci]):
                    nc.tensor.matmul(plT[:], lhsT=RW2[o][:], rhs=rh[:],
                                     start=(i == 0), stop=(i == nmm - 1))
                    i += 1
            # logits = hi rows + lo rows (rows 16..31 are the zero gap);
            # two PSUM operands in one op are illegal, so ACT stages the lo rows
            lgTl = rsb.tile([E, cs], F32, tag="lgTl")
            nc.scalar.copy(lgTl[:], plT[2 * E:3 * E, :])
            lgT = rsb.tile([E, cs], F32, tag="lgT")
            nc.vector.tensor_add(lgT[:], plT[0:E, :], lgTl[:])
            # back to token-major [128, lt*E]: plain K=16 matmuls against a
            # 16x16 identity compute lgT_slice.T (transpose_mode with a
            # 16-partition input crashes the device)
            pl = rpsum.tile([P, LE], F32, tag="rpsum")
            for lt in range(TILES_PER_CH):
                nc.tensor.matmul(pl[:, lt * E:(lt + 1) * E],
                                 lhsT=lgT[:, lt * P:(lt + 1) * P],
                                 rhs=ident[:E, :E],
                                 start=True, stop=True)
            lgb = rsb.tile([P, LE], F32, tag="lgb")
            rbb3 = RBB[:].rearrange("p (l e) -> p l e", l=1).to_broadcast(
                [P, TILES_PER_CH, E])
            nc.vector.tensor_tensor(
                lgb[:].rearrange("p (l e) -> p l e", e=E),
                pl[:].rearrange("p (l e) -> p l e", e=E), rbb3,
                op=mybir.AluOpType.add)
            lg3 = lgb[:].rearrange("p (l e) -> p l e", e=E)
            mx = rsb.tile([P, TILES_PER_CH], F32, tag="mx")
            nc.vector.reduce_max(mx[:], lg3, axis=mybir.AxisListType.X)
            mxb = mx[:].rearrange("p (l e) -> p l e", e=1).to_broadcast(
                [P, TILES_PER_CH, E])
            lgs = rsb.tile([P, LE], F32, tag="lgs")
            nc.vector.tensor_sub(
                lgs[:].rearrange("p (l e) -> p l e", e=E), lg3, mxb)
            ex = rsb.tile([P, LE], F32, tag="ex")
            nc.scalar.activation(ex[:], lgs[:],
                                 mybir.ActivationFunctionType.Exp)
            den = rsb.tile([P, TILES_PER_CH], F32, tag="den")
            nc.vector.reduce_sum(den[:],
                                 ex[:].rearrange("p (l e) -> p l e", e=E),
                                 axis=mybir.AxisListType.X)
            rden = rsb.tile([P, TILES_PER_CH], F32, tag="rden")
            nc.vector.reciprocal(rden[:], den[:])
            t8b = rsb.tile([P, 8 * TILES_PER_CH], F32, tag="t8b")
            for lt in range(TILES_PER_CH):
                nc.vector.max(out=t8b[:, lt * 8:(lt + 1) * 8],
                              in_=lgb[:, lt * E:(lt + 1) * E])
            thrb = t8b[:].rearrange("p (l k) -> p l k", k=8)[:, :, 3:4] \
                .to_broadcast([P, TILES_PER_CH, E])
            mask = rsb.tile([P, LE], F32, tag="mask")
            nc.vector.tensor_tensor(
                mask[:].rearrange("p (l e) -> p l e", e=E), lg3, thrb,
                op=mybir.AluOpType.is_ge)
            spm = rsb.tile([P, LE], F32, tag="spm")
            nc.vector.tensor_mul(spm[:], ex[:], mask[:])
            spv = rsb.tile([P, LE], F32, tag="spv")
            rdenb = rden[:].rearrange("p (l e) -> p l e", e=1).to_broadcast(
                [P, TILES_PER_CH, E])
            nc.vector.tensor_tensor(
                spv[:].rearrange("p (l e) -> p l e", e=E),
                spm[:].rearrange("p (l e) -> p l e", e=E), rdenb,
                op=mybir.AluOpType.mult)
            for lt in range(TILES_PER_CH):
                t0 = c0 + lt * P
                pt = tpsum.tile([E, P], F32, tag="tpsum",
                                name=f"pt{tci}_{lt}")
                nc.tensor.transpose(pt[:], spv[:, lt * E:(lt + 1) * E],
                                    ident[:])
                nc.vector.tensor_copy(spT[:, t0:t0 + P], pt[:])
            # gate rows to DRAM, then broadcast-load one [P, TCH] row/expert
            # (on the ACT HWDGE queue, so they don't queue behind W loads)
            nc.scalar.dma_start(spd[:, ccols], spT[:, ccols])
            sreps = []
            for e in range(E):
                srep = srpool.tile([P, cs], BF16, tag="srep",
                                   name=f"srep{e}_{tci}")
                nc.scalar.dma_start(srep[:],
                                    spd[e:e + 1, ccols].to_broadcast([P, cs]))
                sreps.append(srep)
            srep_by_chunk[tci] = sreps

        def main_chunk(tci):
            c0, cs = CH_STARTS[tci], CH_SIZES[tci]
            ccols = slice(c0, c0 + cs)
            sreps = srep_by_chunk[tci]
            # ---------------- expert accumulation (o-outer) ----------------
            ps = [mpsum.tile([P, cs], F32, tag=f"ps{a}",
                             name=f"ps{a}_{tci}") for a in range(4)]
            for a in range(4):
                nc.tensor.matmul(ps[a][:], lhsT=Bc[:, a * P:(a + 1) * P],
                                 rhs=spT[:, ccols], start=True, stop=False)
            for o in range(OT):
                for e in range(E):
                    r = rspool.tile([P, cs], BF16, tag="rs",
                                    name=f"rs{o}_{e}_{tci}")
                    nc.vector.tensor_mul(r[:], X[o][tci][:], sreps[e][:])
                    for a in range(4):
                        nc.tensor.matmul(
                            ps[a][:],
                            lhsT=W[o][:, e * A2 + a * P: e * A2 + (a + 1) * P],
                            rhs=r[:],
                            start=False,
                            stop=(o == OT - 1 and e == E - 1),
                        )
            for a in range(2):
                ob = opool.tile([P, cs], F32, tag="ob")
                nc.scalar.copy(ob[:], ps[a][:])
                nc.sync.dma_start(outT_d[a * P:(a + 1) * P, ccols], ob[:])
            for a in range(2, 4):
                th = opool.tile([P, cs], F32, tag="th")
                nc.scalar.activation(th[:], ps[a][:],
                                     mybir.ActivationFunctionType.Tanh)
                ob = opool.tile([P, cs], F32, tag="ob")
                nc.vector.tensor_scalar(ob[:], th[:], LOG_STD_SCALE,
                                        LOG_STD_SHIFT,
                                        op0=mybir.AluOpType.mult,
                                        op1=mybir.AluOpType.add)
                nc.sync.dma_start(outT_d[a * P:(a + 1) * P, ccols], ob[:])

        for i in range(NTC):
            router_chunk(i)
            main_chunk(i)

    nc.compile()
    return nc


def _host_prep(inputs, ncores=NCORES):
    x = np.asarray(inputs["x"], np.float32)
    rw = np.asarray(inputs["router_w"], np.float32)
    rb = np.asarray(inputs["router_b"], np.float32)
    mw = np.asarray(inputs["mean_w"], np.float32)
    mb = np.asarray(inputs["mean_b"], np.float32)
    lw = np.asarray(inputs["log_std_w"], np.float32)
    lb = np.asarray(inputs["log_std_b"], np.float32)

    B = x.shape[0]
    T = B // ncores

    # wcat[o, e*A2 + a] = (mean|log_std)_w[e, a, o]
    wc = np.concatenate([mw.transpose(0, 2, 1), lw.transpose(0, 2, 1)], axis=2)
    wcat = np.ascontiguousarray(wc.transpose(1, 0, 2)).reshape(OBS, E * A2)
    wcat = wcat.astype(NP_BF16)
    bcat = np.concatenate([mb, lb], axis=1).astype(NP_BF16)
    rwT = np.ascontiguousarray(rw.T).astype(np.float32)
    rbv = rb.reshape(1, E).astype(np.float32)

    shards = x.reshape(ncores, T, OBS)
    in_maps = []
    for c in range(ncores):
        xTf = np.ascontiguousarray(shards[c].T.astype(np.float32))
        xh = xTf.astype(NP_BF16)
        xl = (xTf - xh.astype(np.float32)).astype(NP_BF16)
        in_maps.append({"xh": xh, "xl": xl, "wcat": wcat, "bcat": bcat,
                        "rwT": rwT, "rb": rbv})
    return in_maps, T


_NC_CACHE = {}


def _get_nc(T):
    if T not in _NC_CACHE:
        _NC_CACHE[T] = build_nc(T)
    return _NC_CACHE[T]


def run_sharded(inputs, trace=False):
    """Returns ((mean, log_std), BassKernelResults)."""
    in_maps, T = _host_prep(inputs)
    nc = _get_nc(T)
    res = run_bass_kernel_spmd(nc, in_maps, list(range(NCORES)), trace=trace)
    outs = [res.results[c]["outT"] for c in range(NCORES)]
    mean = np.concatenate([o[:ACT_DIM].T for o in outs], axis=0)
    log_std = np.concatenate([o[ACT_DIM:].T for o in outs], axis=0)
    return (np.ascontiguousarray(mean, dtype=np.float32),
            np.ascontiguousarray(log_std, dtype=np.float32)), res


def kernel(**inputs):
    (mean, log_std), _ = run_sharded(inputs, trace=False)
    return mean, log_std

